# revision 1
# baseline (speedup 1.0000x reference)
"""Trainium2 Bass kernel for nn_Attention_30468497997979.

Reference computation (per batch b of 8):
    X = hidden_states[b,:,0,:]              # (C=768, S=384)
    Q/K/V = W @ X + b                       # 1x1 conv == channel matmul
    per head h (12 heads, head dim 64, channel c = d*12 + h):
        scores = (Q_h^T K_h) / 8, mask q>k, softmax over k
        attn_h = V_h @ softmax
    out = Wo @ concat_heads(attn)           # channel c = h*64 + d

Sharding: pure data-parallel, one batch per NeuronCore (8 cores).

Per-core kernel layout choices:
  - Host pre-permutes W_{q,k,v} rows to head-major channel order
    (c' = h*64 + d) and transposes all weights to [c_in, c_out] so the
    contraction dim lands on SBUF partitions. 1/sqrt(d) folded into Wq/bq.
  - scores are computed transposed ([k, q] with keys on partitions):
    lhsT = K_h k-chunk, rhs = Q_h. Softmax needs no max-subtraction
    (scores are O(1); masked entries get -1e4 -> exp == 0).
  - V is projected directly in transposed [s, c'] layout (lhsT = X chunk,
    rhs = WvT), so the attn@V matmul contracts over k on partitions with
    no on-chip transposes anywhere.
  - The softmax denominator is fused into the attn@V matmul as an extra
    ones-column appended to each V tile (psum row 64 = column sums).
  - The V bias commutes through attention exactly (softmax rows sum to 1)
    and is folded on the host into an output-projection bias Wo @ bv.
  - Normalization is deferred: denominator rows collect in SBUF, 1/sums
    runs as a few batched DVE reciprocals, and each row is broadcast
    across partitions with a K=1 PE matmul before a DVE multiply.
  - Matmul data is bf16 (the PE streams 1 output row/cycle vs ~2 for
    fp32r; half the HBM traffic). PSUM accumulation stays fp32; measured
    ~4e-3 relative error vs the fp32 reference. Set QK_DT/V_DT/O_DT to
    "f32r" for a ~2.7e-4-error, ~16%-slower variant.
"""

import numpy as np

B, C, S, H, D = 8, 768, 384, 12, 64
NC_CHUNKS = C // 128  # 6
NEG = -10000.0

# matmul dtypes per stage: "f32r" (full fp32 data, ~2 PE cycles/row) or
# "bf16" (1 cycle/row, half the DMA bytes, ~1e-3 rel err)
QK_DT = "bf16"   # x, Wq, Wk, q, k (score path)
V_DT = "bf16"    # x2, Wv, vt, e (attn@V path)
O_DT = "bf16"    # Wo, attn (output projection)

_STATE = {}


# --------------------------------------------------------------------------
# Workaround: this walrus build rejects the multi-wait InstDrain that
# TileContext emits at exit ("Too many sync wait commands"). Split the
# drain's sem waits onto standalone sync-engine wait instructions.
def _patch_walrus_ldw_opt():
    """Enable walrus's load-weight pipelining (ldw-opt): overlaps each
    matmul's LDWEIGHTS with the previous matmul's execution."""
    import os
    import concourse.bass_utils as bu

    if os.environ.get("KERNEL_LDW_OPT") != "1":
        return
    if getattr(bu, "_ldw_opt_patch", False):
        return
    orig = bu.run_command

    def patched(argv, **kwargs):
        argv = [
            a.replace("--enable-ldw-opt=false", "--enable-ldw-opt=true")
            if isinstance(a, str)
            else a
            for a in argv
        ]
        return orig(argv, **kwargs)

    bu.run_command = patched
    bu._ldw_opt_patch = True


def _patch_tile_drain():
    import concourse.tile as tile_mod
    from concourse.vector_clock import ScopedClock
    from bass_rust import SyncInfo

    if getattr(tile_mod.TileContext, "_drain_split_patch", False):
        return

    def _drain_and_barrier_split(self, tick_clock, wait_clock):
        nc = self.nc
        assert self.sems is not None
        handles = {}
        for h in self.sems.allocated().values():
            handles[h.num] = h
            handles[h.name] = h

        probe = nc.sync.nop()
        wait_clock.add_sem_waits(
            probe.ins, ScopedClock({None: tick_clock.global_clock})
        )
        waits = list(probe.ins.sync_info.on_wait)
        probe.ins.sync_info = SyncInfo(on_wait=[], on_update=[])
        for w in waits:
            h = handles.get(w.id) or handles.get(w.ant_name)
            if h is not None:
                nc.sync.wait_ge(h, w.wait_value)
            else:
                n2 = nc.sync.nop()
                n2.ins.sync_info = SyncInfo(on_wait=[w], on_update=[])

        drain_inst = nc.sync.drain()
        wait_clock.add_sem_waits(
            drain_inst.ins, ScopedClock({None: tick_clock.global_clock})
        )
        if list(drain_inst.ins.sync_info.on_wait):
            drain_inst.ins.sync_info = SyncInfo(on_wait=[], on_update=[])

        nc.all_engine_barrier()
        popped = nc._tile_sem_poison_stack.pop()
        assert popped is self._sem_poison
        nc.clear_and_free_semaphores(list(self.sems.allocated().values()))
        nc.all_engine_barrier()

        # This walrus codegen supports at most ONE sem wait per
        # instruction. Move extra waits onto same-engine nop carriers
        # inserted just before the instruction (engine queues execute in
        # order, so the semantics are identical).
        import concourse.mybir as mybir

        k = 0
        for f in nc.m.functions:
            for bb in f.blocks:
                new_insts = []
                for inst in bb.instructions:
                    si = inst.sync_info
                    waits = list(si.on_wait) if si else []
                    if len(waits) > 1:
                        for w in waits[:-1]:
                            nop = mybir.InstNoOp(name=f"I-wsplit-{k}")
                            k += 1
                            nop.engine = inst.engine
                            nop.sync_info = SyncInfo(on_wait=[w], on_update=[])
                            nc.register_instruction(nop)
                            new_insts.append(nop)
                        inst.sync_info = SyncInfo(
                            on_wait=[waits[-1]], on_update=list(si.on_update)
                        )
                    new_insts.append(inst)
                bb.instructions = new_insts

    tile_mod.TileContext._drain_and_barrier = _drain_and_barrier_split
    tile_mod.TileContext._drain_split_patch = True


# --------------------------------------------------------------------------
def _build_nc(use_f32r=True):
    import concourse.bass as bass
    import concourse.mybir as mybir
    import concourse.tile as tile

    _patch_tile_drain()
    _patch_walrus_ldw_opt()

    f32 = mybir.dt.float32
    f32r = mybir.dt.float32r
    bf16 = mybir.dt.bfloat16
    Ident = mybir.ActivationFunctionType.Identity
    Exp = mybir.ActivationFunctionType.Exp

    dmap = {"f32r": f32r, "bf16": bf16, "f32": f32}
    dt_qk, dt_v, dt_o = dmap[QK_DT], dmap[V_DT], dmap[O_DT]
    dtm = f32r if use_f32r else f32

    nc = bass.Bass()
    x_d = nc.dram_tensor("x", [C, S], dt_qk, kind="ExternalInput")
    x2_d = (
        nc.dram_tensor("x2", [C, S], dt_v, kind="ExternalInput")
        if dt_v != dt_qk
        else None
    )
    wq_d = nc.dram_tensor("wqt", [C, C], dt_qk, kind="ExternalInput")
    wk_d = nc.dram_tensor("wkt", [C, C], dt_qk, kind="ExternalInput")
    wv_d = nc.dram_tensor("wvt", [C, C], dt_v, kind="ExternalInput")
    wo_d = nc.dram_tensor("wot", [C, C], dt_o, kind="ExternalInput")
    bq_d = nc.dram_tensor("bq", [C, 1], f32, kind="ExternalInput")
    bk_d = nc.dram_tensor("bk", [C, 1], f32, kind="ExternalInput")
    # V-bias folded through attention (softmax rows sum to 1) into a
    # host-precomputed output bias: obias = Wo @ bv_headmajor
    ob_d = nc.dram_tensor("obias", [C, 1], f32, kind="ExternalInput")
    # diagonal 128x128 triangle blocks of the [k, q] mask, stacked
    mask_d = nc.dram_tensor("maskd", [S, 128], f32, kind="ExternalInput")
    konst_d = nc.dram_tensor("konst", [128, D], dtm, kind="ExternalInput")
    konstv_d = (
        nc.dram_tensor("konstv", [128, D], dt_v, kind="ExternalInput")
        if dt_v != dtm
        else None
    )
    y_d = nc.dram_tensor("y", [C, S], f32, kind="ExternalOutput")

    with tile.TileContext(nc) as tc:
        with (
            tc.tile_pool(name="persist", bufs=1) as persist,
            tc.tile_pool(name="epool", bufs=9) as epool,
            tc.tile_pool(name="small", bufs=4) as small,
            tc.tile_pool(name="psA", bufs=2, space="PSUM") as psA,
            tc.tile_pool(name="psS", bufs=4, space="PSUM") as psS,
            tc.tile_pool(name="psV", bufs=1, space="PSUM") as psV,
            tc.tile_pool(name="psR", bufs=1, space="PSUM") as psR,
        ):
            # ---- loads -------------------------------------------------
            # x and wv chunks alternate across the Sync and GpSimd queues
            # (first compute needs x0+wv0 ASAP); wq/wo load as single big
            # DMAs on the Activation queue (idle at start), wk on GpSimd.
            xt = [
                persist.tile([128, S], dt_qk, tag=f"x{i}", name=f"x{i}")
                for i in range(NC_CHUNKS)
            ]
            wv_sb = [
                persist.tile([128, C], dt_v, tag=f"wv{i}", name=f"wv{i}")
                for i in range(NC_CHUNKS)
            ]
            if x2_d is not None:
                xv = [
                    persist.tile([128, S], dt_v, tag=f"xv{i}", name=f"xv{i}")
                    for i in range(NC_CHUNKS)
                ]
            else:
                xv = xt
            for i in range(NC_CHUNKS):
                ex, ev = (nc.sync, nc.gpsimd) if i % 2 == 0 else (nc.gpsimd, nc.sync)
                ev.dma_start(wv_sb[i][:], wv_d[i * 128 : (i + 1) * 128, :])
                if x2_d is not None:
                    ex.dma_start(xv[i][:], x2_d[i * 128 : (i + 1) * 128, :])
                ex.dma_start(xt[i][:], x_d[i * 128 : (i + 1) * 128, :])

            def load_w_mono(dram, tag, eng, dt_):
                t = persist.tile([128, NC_CHUNKS, C], dt_, tag=tag, name=tag)
                eng.dma_start(
                    t[:], dram.rearrange("(cc p) c -> p cc c", p=128)
                )
                return [t[:, i, :] for i in range(NC_CHUNKS)]

            wq_sb = load_w_mono(wq_d, "wq", nc.scalar, dt_qk)
            wk_sb = load_w_mono(wk_d, "wk", nc.gpsimd, dt_qk)
            wo_sb = load_w_mono(wo_d, "wo", nc.scalar, dt_o)

            def load_b(dram, tag):
                tiles = []
                for i in range(NC_CHUNKS):
                    t = persist.tile([128, 1], f32, tag=f"{tag}{i}", name=f"{tag}{i}")
                    nc.gpsimd.dma_start(t[:], dram[i * 128 : (i + 1) * 128, :])
                    tiles.append(t)
                return tiles

            bq_sb = load_b(bq_d, "bq")
            bk_sb = load_b(bk_d, "bk")
            ob_sb = load_b(ob_d, "ob")

            mask_sb = []
            for kc in range(3):
                t = persist.tile([128, 128], f32, tag=f"mask{kc}", name=f"mask{kc}")
                nc.gpsimd.dma_start(t[:], mask_d[kc * 128 : (kc + 1) * 128, :])
                mask_sb.append(t)

            # ---- V projection, transposed: vt[sq][s, h, 0:64] = V'[c', s]^T
            # col 64 of each head slot = 1.0 (fused denominator column)
            vt = []
            for sq in range(3):
                t = persist.tile([128, H, D + 1], dt_v, tag=f"vt{sq}", name=f"vt{sq}")
                kd = konstv_d if konstv_d is not None else konst_d
                nc.gpsimd.dma_start(
                    t[:, :, D : D + 1],
                    kd[:, 0:H].rearrange("p (h o) -> p h o", o=1),
                )
                vt.append(t)
            for sq in range(3):
                for half in range(2):
                    ps = psA.tile([128, S], f32, tag="proj", name="proj")
                    for cc in range(NC_CHUNKS):
                        nc.tensor.matmul(
                            ps[:],
                            xv[cc][:, sq * 128 : (sq + 1) * 128],
                            wv_sb[cc][:, half * 384 : (half + 1) * 384],
                            start=(cc == 0),
                            stop=(cc == NC_CHUNKS - 1),
                        )
                    nc.vector.tensor_copy(
                        vt[sq][:, half * 6 : (half + 1) * 6, 0:D],
                        ps[:].rearrange("p (h d) -> p h d", d=D),
                    )

            # ---- Q/K projections (head-major rows; scale folded into Wq)
            q_sb = [
                persist.tile([128, S], dt_qk, tag=f"q{oc}", name=f"q{oc}")
                for oc in range(NC_CHUNKS)
            ]
            k_sb = [
                persist.tile([128, S], dt_qk, tag=f"k{oc}", name=f"k{oc}")
                for oc in range(NC_CHUNKS)
            ]

            def proj(oc, w_tiles, bias, out):
                ps = psA.tile([128, S], f32, tag="proj", name="proj")
                for cc in range(NC_CHUNKS):
                    nc.tensor.matmul(
                        ps[:],
                        w_tiles[cc][:, oc * 128 : (oc + 1) * 128],
                        xt[cc],
                        start=(cc == 0),
                        stop=(cc == NC_CHUNKS - 1),
                    )
                nc.scalar.activation(out[:], ps[:], Ident, bias=bias[:])

            # ---- attention -------------------------------------------
            # per head: scores^T -> masked exp -> attn@V with fused
            # denominator row. Normalization deferred: unnormalized attn
            # and the denominator rows land in SBUF; reciprocals run
            # batched per 6-head group, broadcast via one strided DMA.
            attn_un = [
                persist.tile([128, S], f32, tag=f"au{oc}", name=f"au{oc}")
                for oc in range(NC_CHUNKS)
            ]
            attn_sb = [
                persist.tile([128, S], dt_o, tag=f"at{oc}", name=f"at{oc}")
                for oc in range(NC_CHUNKS)
            ]
            sums_sb = persist.tile([H, S], f32, tag="sums", name="sums")
            nc.vector.memset(sums_sb[:], 1.0)
            rinv_sb = persist.tile([H, S], f32, tag="rinv", name="rinv")
            rinv_r = persist.tile([1, H, S], dtm, tag="rinvr", name="rinvr")
            ones_sb = persist.tile([1, D], dtm, tag="ones", name="ones")
            nc.gpsimd.dma_start(ones_sb[:], konst_d[0:1, :])

            def head_scores(h):
                oc, prow = h // 2, (h % 2) * D
                Qh = q_sb[oc][prow : prow + D, :]
                Kh = k_sb[oc][prow : prow + D, :]
                e_tiles = []
                for kc in range(3):
                    ncols = 128 * (kc + 1)
                    ps_s = psS.tile([128, S], f32, tag="sc", name="sc")
                    nc.tensor.matmul(
                        ps_s[:, 0:ncols],
                        Kh[:, kc * 128 : (kc + 1) * 128],
                        Qh[:, 0:ncols],
                        start=True,
                        stop=True,
                    )
                    # causal mask: only the diagonal block needs masking
                    nc.vector.tensor_add(
                        ps_s[:, kc * 128 : ncols],
                        ps_s[:, kc * 128 : ncols],
                        mask_sb[kc][:],
                    )
                    e = epool.tile([128, S], dt_v, tag="e", name="e")
                    nc.scalar.activation(e[:, 0:ncols], ps_s[:, 0:ncols], Exp)
                    e_tiles.append(e)
                return e_tiles

            def head_av(h, e_tiles):
                # attn@V; accumulate widest first so every element's first
                # write carries the start flag
                oc, prow = h // 2, (h % 2) * D
                ps_av = psV.tile([D + 1, S], f32, tag="av", name="av")
                for step, kc in enumerate([2, 1, 0]):
                    ncols = 128 * (kc + 1)
                    nc.tensor.matmul(
                        ps_av[:, 0:ncols],
                        vt[kc][:, h, :],
                        e_tiles[kc][:, 0:ncols],
                        start=(step == 0),
                        stop=(step == 2),
                        skip_group_check=True,
                    )
                st = small.tile([1, S], f32, tag="st", name="st")
                nc.vector.tensor_copy(st[:], ps_av[D : D + 1, :])
                nc.gpsimd.dma_start(sums_sb[h : h + 1, :], st[:])
                nc.scalar.copy(attn_un[oc][prow : prow + D, :], ps_av[0:D, :])

            def normalize_group(h0, h1):
                # rinv = 1/sums for heads [h0,h1) — one fused-NR DVE op on
                # h1-h0 lanes; a tiny DMA repacks the rows into one
                # partition's free dim (f32r view) so a K=1 matmul can
                # broadcast each row across 64 partitions.
                sl = slice(h0, h1)
                # compute engines need 32-aligned partition starts: run the
                # reciprocal over all 12 rows from partition 0 (later rows
                # recompute to the same values; unready rows are never read)
                nc.vector.reciprocal(rinv_sb[:, :], sums_sb[:, :])
                # repack rows into one partition (f32r view), then K=1
                # matmuls broadcast each row across 64 partitions
                nc.sync.dma_start(rinv_r[0:1, sl, :], rinv_sb[sl, :].bitcast(dtm))
                for h in range(h0, h1):
                    oc, prow = h // 2, (h % 2) * D
                    ps_r = psR.tile([D, S], f32, tag="rb", name="rb")
                    nc.tensor.matmul(
                        ps_r[:], ones_sb[:], rinv_r[0:1, h, :],
                        start=True, stop=True,
                    )
                    nc.vector.tensor_mul(
                        attn_sb[oc][prow : prow + D, :],
                        attn_un[oc][prow : prow + D, :],
                        ps_r[:],
                    )

            # two-stage software pipeline across heads: head h+1's score
            # matmuls are queued before head h's attn@V, so the PE has real
            # work during the mask->exp latency instead of a static stall
            pending = None
            for oc in range(NC_CHUNKS):
                proj(oc, wq_sb, bq_sb[oc], q_sb[oc])
                proj(oc, wk_sb, bk_sb[oc], k_sb[oc])
                for h in (2 * oc, 2 * oc + 1):
                    e_tiles = head_scores(h)
                    if pending is not None:
                        head_av(*pending)
                    pending = (h, e_tiles)
                    if h - 1 == 5:
                        normalize_group(0, 6)
                    if h - 1 == 9:
                        normalize_group(6, 10)
            head_av(*pending)
            normalize_group(10, 12)

            # ---- output projection (bias = host-folded Wo @ bv) --------
            for oc in range(NC_CHUNKS):
                ps = psA.tile([128, S], f32, tag="proj", name="proj")
                for cc in range(NC_CHUNKS):
                    nc.tensor.matmul(
                        ps[:],
                        wo_sb[cc][:, oc * 128 : (oc + 1) * 128],
                        attn_sb[cc],
                        start=(cc == 0),
                        stop=(cc == NC_CHUNKS - 1),
                    )
                ot = epool.tile([128, S], f32, tag="o", name="o", bufs=3)
                nc.scalar.activation(ot[:], ps[:], Ident, bias=ob_sb[oc][:])
                nc.sync.dma_start(y_d[oc * 128 : (oc + 1) * 128, :], ot[:])

    return nc


def _get_nc():
    if "nc" not in _STATE:
        _STATE["nc"] = _build_nc()
    return _STATE["nc"]


# --------------------------------------------------------------------------
def _np_dt(name):
    if name == "bf16":
        import ml_dtypes

        return ml_dtypes.bfloat16
    return np.float32


def _prep_maps(inputs):
    hs = np.asarray(inputs["hidden_states"], dtype=np.float32)
    Wq = np.asarray(inputs["Wq"], dtype=np.float32)
    bq = np.asarray(inputs["bq"], dtype=np.float32)
    Wk = np.asarray(inputs["Wk"], dtype=np.float32)
    bk = np.asarray(inputs["bk"], dtype=np.float32)
    Wv = np.asarray(inputs["Wv"], dtype=np.float32)
    bv = np.asarray(inputs["bv"], dtype=np.float32)
    Wo = np.asarray(inputs["Wo"], dtype=np.float32)

    # head-major channel permutation: c' = h*64 + d  <-  c = d*12 + h
    idx = (np.arange(H)[:, None] + np.arange(D)[None, :] * H).reshape(C)
    scale = float(D) ** -0.5

    dqk, dv, do = _np_dt(QK_DT), _np_dt(V_DT), _np_dt(O_DT)
    wqt = np.ascontiguousarray((scale * Wq[idx, :]).T).astype(dqk)
    wkt = np.ascontiguousarray(Wk[idx, :].T).astype(dqk)
    wvt = np.ascontiguousarray(Wv[idx, :].T).astype(dv)
    wot = np.ascontiguousarray(Wo.T).astype(do)
    bq2 = np.ascontiguousarray((scale * bq[idx]).reshape(C, 1))
    bk2 = np.ascontiguousarray(bk[idx].reshape(C, 1))

    # mask[k, q] = NEG where k < q; per k-chunk only the diagonal
    # triangle block needs masking
    blk = np.triu(np.full((128, 128), NEG, dtype=np.float32), 1)
    maskd = np.ascontiguousarray(np.tile(blk, (3, 1)))
    konst = np.ones((128, D), dtype=np.float32)
    # V-bias folded through attention (softmax rows sum to 1):
    # attn' = attn_nobias' + bv[idx], so out += Wo @ bv[idx]
    obias = np.ascontiguousarray((Wo @ bv[idx]).reshape(C, 1).astype(np.float32))

    shared = {
        "wqt": wqt, "wkt": wkt, "wvt": wvt, "wot": wot,
        "bq": bq2, "bk": bk2, "obias": obias, "maskd": maskd,
        "konst": konst,
    }
    if V_DT != "f32r":
        shared["konstv"] = konst.astype(dv)
    maps = []
    for b in range(B):
        xb = np.ascontiguousarray(hs[b, :, 0, :])
        m = {"x": xb.astype(dqk), **shared}
        if V_DT != QK_DT:
            m["x2"] = xb.astype(dv)
        maps.append(m)
    return maps


def _run(inputs, trace=False, **kwargs):
    from concourse.bass_utils import run_bass_kernel_spmd

    nc = _get_nc()
    in_maps = _prep_maps(inputs)
    res = run_bass_kernel_spmd(
        nc, in_maps, core_ids=list(range(B)), trace=trace, **kwargs
    )
    out = np.stack([res.results[b]["y"] for b in range(B)], axis=0)
    return out.reshape(B, C, 1, S).astype(np.float32), res


def kernel(**inputs):
    out, _ = _run(inputs, trace=False)
    return out



# revision 9
# speedup vs baseline: 1.3772x; 1.3772x over previous
"""Trainium2 Bass kernel for nn_Attention_30468497997979.

Reference computation (per batch b of 8):
    X = hidden_states[b,:,0,:]              # (C=768, S=384)
    Q/K/V = W @ X + b                       # 1x1 conv == channel matmul
    per head h (12 heads, head dim 64, channel c = d*12 + h):
        scores = (Q_h^T K_h) / 8, mask (keys k < q masked), softmax over k
        attn_h = V_h @ softmax
    out = Wo @ concat_heads(attn)           # channel c = h*64 + d

Sharding: pure data-parallel, one batch per NeuronCore (8 cores).

Per-core kernel design (v2 — rebalanced engines, consolidated DMAs):
  - Host pre-permutes W_{q,k,v} rows to head-major channel order
    (c' = h*64 + d) and transposes all weights to [c_in, c_out].
    1/sqrt(d) folded into Wq/bq; V bias folded through attention into an
    output bias Wo @ bv (softmax rows sum to 1). All matmul data bf16.
  - Every dma_start costs ~0.7us of issuing-engine time regardless of
    size, so inputs load as ONE monolithic DMA per tensor ([128, 6, *]
    rearranged), with wq split in two so Q-proj of chunk 0 starts early.
    Small constants (biases, 0/1 triangle mask) ride in one packed
    block. All ones-constants are memset on-chip.
  - scores are computed transposed ([k, q], keys on partitions):
    lhsT = K_h k-chunk, rhs = Q_h. Causal trimming: k-chunk kc only
    needs q-columns 0..(kc+1)*128. Per head two PSUM tiles:
    A = [kc0 q0:128 | kc1 q0:256] (one bank), B = [kc2 q0:384].
  - softmax needs no max-subtraction (scores are O(1)). The mask is
    applied multiplicatively AFTER exp: exp(s+m) = exp(s)*{0,1}, so the
    Scalar engine exps straight out of PSUM (2 calls/head) and the
    GpSimd engine (no PSUM port) multiplies the three diagonal
    [128,128] sub-blocks by a 0/1 triangle in bf16 SBUF.
  - attn@V contracts over k on partitions (lhsT = per-head V^T tile
    with a fused ones-column computing the softmax denominator as PSUM
    row 64). DVE copies rows 0:64 to an unnormalized-attn tile (head
    parity picks partition half), Scalar copies the denominator row
    into a staging row.
  - Normalization: batched DVE reciprocal over [3n,128]-shaped sums
    (reciprocal cost is free-dim bound), one DMA repack per group, then
    per-CHUNK (2 heads at once) a K=2 matmul broadcasts both heads'
    1/sum rows across the right partition halves (lhsT is a 0/1
    selector), and one DVE multiply normalizes the whole [128,384]
    chunk. Groups (heads 0-7, 8-11) keep the tail short.
  - Engine budget: PE ~34us (bound), Scalar = exps+sums+output bias,
    DVE = bias-adds/copies/reciprocal/normalize, GpSimd = masks+DMAs.
"""

import numpy as np

B, C, S, H, D = 8, 768, 384, 12, 64
NC_CHUNKS = C // 128  # 6

_STATE = {}


# --------------------------------------------------------------------------
# Workaround: this walrus build rejects the multi-wait InstDrain that
# TileContext emits at exit ("Too many sync wait commands"). Split the
# drain's sem waits onto standalone sync-engine wait instructions.
def _patch_walrus_ldw_opt():
    """Enable walrus's load-weight pipelining (ldw-opt): overlaps each
    matmul's LDWEIGHTS with the previous matmul's execution."""
    import os
    import concourse.bass_utils as bu

    if os.environ.get("KERNEL_LDW_OPT") != "1":
        return
    if getattr(bu, "_ldw_opt_patch", False):
        return
    orig = bu.run_command

    def patched(argv, **kwargs):
        argv = [
            a.replace("--enable-ldw-opt=false", "--enable-ldw-opt=true")
            if isinstance(a, str)
            else a
            for a in argv
        ]
        return orig(argv, **kwargs)

    bu.run_command = patched
    bu._ldw_opt_patch = True


def _patch_tile_drain():
    import concourse.tile as tile_mod
    from concourse.vector_clock import ScopedClock
    from bass_rust import SyncInfo

    if getattr(tile_mod.TileContext, "_drain_split_patch", False):
        return

    def _drain_and_barrier_split(self, tick_clock, wait_clock):
        nc = self.nc
        assert self.sems is not None
        handles = {}
        for h in self.sems.allocated().values():
            handles[h.num] = h
            handles[h.name] = h

        probe = nc.sync.nop()
        wait_clock.add_sem_waits(
            probe.ins, ScopedClock({None: tick_clock.global_clock})
        )
        waits = list(probe.ins.sync_info.on_wait)
        probe.ins.sync_info = SyncInfo(on_wait=[], on_update=[])
        for w in waits:
            h = handles.get(w.id) or handles.get(w.ant_name)
            if h is not None:
                nc.sync.wait_ge(h, w.wait_value)
            else:
                n2 = nc.sync.nop()
                n2.ins.sync_info = SyncInfo(on_wait=[w], on_update=[])

        drain_inst = nc.sync.drain()
        wait_clock.add_sem_waits(
            drain_inst.ins, ScopedClock({None: tick_clock.global_clock})
        )
        if list(drain_inst.ins.sync_info.on_wait):
            drain_inst.ins.sync_info = SyncInfo(on_wait=[], on_update=[])

        nc.all_engine_barrier()
        popped = nc._tile_sem_poison_stack.pop()
        assert popped is self._sem_poison
        nc.clear_and_free_semaphores(list(self.sems.allocated().values()))
        nc.all_engine_barrier()

        # This walrus codegen supports at most ONE sem wait per
        # instruction. Move extra waits onto same-engine nop carriers
        # inserted just before the instruction (engine queues execute in
        # order, so the semantics are identical).
        import concourse.mybir as mybir

        k = 0
        for f in nc.m.functions:
            for bb in f.blocks:
                new_insts = []
                for inst in bb.instructions:
                    si = inst.sync_info
                    waits = list(si.on_wait) if si else []
                    if len(waits) > 1:
                        for w in waits[:-1]:
                            nop = mybir.InstNoOp(name=f"I-wsplit-{k}")
                            k += 1
                            nop.engine = inst.engine
                            nop.sync_info = SyncInfo(on_wait=[w], on_update=[])
                            nc.register_instruction(nop)
                            new_insts.append(nop)
                        inst.sync_info = SyncInfo(
                            on_wait=[waits[-1]], on_update=list(si.on_update)
                        )
                    new_insts.append(inst)
                bb.instructions = new_insts

    tile_mod.TileContext._drain_and_barrier = _drain_and_barrier_split
    tile_mod.TileContext._drain_split_patch = True


# --------------------------------------------------------------------------
def _build_nc():
    import concourse.bass as bass
    import concourse.mybir as mybir
    import concourse.tile as tile

    _patch_tile_drain()
    _patch_walrus_ldw_opt()

    f32 = mybir.dt.float32
    f32r = mybir.dt.float32r
    bf16 = mybir.dt.bfloat16
    Ident = mybir.ActivationFunctionType.Identity
    Copy = mybir.ActivationFunctionType.Copy
    Exp = mybir.ActivationFunctionType.Exp

    nc = bass.Bass()
    x_d = nc.dram_tensor("x", [C, S], bf16, kind="ExternalInput")
    wqa_d = nc.dram_tensor("wqa", [C, 128], bf16, kind="ExternalInput")
    wqb_d = nc.dram_tensor("wqb", [C, C - 128], bf16, kind="ExternalInput")
    wk_d = nc.dram_tensor("wkt", [C, C], bf16, kind="ExternalInput")
    wv_d = nc.dram_tensor("wvt", [C, C], bf16, kind="ExternalInput")
    wo_d = nc.dram_tensor("wot", [C, C], bf16, kind="ExternalInput")
    # packed constants [128, 146] f32:
    #   cols 0:6 bq (col=chunk), 6:12 bk, 12:18 obias (= Wo @ bv'),
    #   cols 18:146 = [128, 256] bf16 = 0/1 lower-triangle (k>=q) twice
    cst_d = nc.dram_tensor("cst", [128, 146], f32, kind="ExternalInput")
    sel_d = nc.dram_tensor("sel", [2, 128], f32, kind="ExternalInput")
    y_d = nc.dram_tensor("y", [C, S], bf16, kind="ExternalOutput")

    with tile.TileContext(nc) as tc:
        with (
            tc.tile_pool(name="persist", bufs=1) as persist,
            tc.tile_pool(name="epool", bufs=6) as epool,
            tc.tile_pool(name="psA", bufs=2, space="PSUM") as psA,
            tc.tile_pool(name="psS", bufs=4, space="PSUM") as psS,
            tc.tile_pool(name="psV", bufs=2, space="PSUM") as psV,
        ):
            # ---- input loads: one monolithic DMA per tensor ----------
            xt = persist.tile([128, NC_CHUNKS, S], bf16, tag="x", name="x")
            nc.sync.dma_start(xt[:], x_d.rearrange("(cc p) s -> p cc s", p=128))
            wv_sb = persist.tile([128, NC_CHUNKS, C], bf16, tag="wv", name="wv")
            nc.sync.dma_start(wv_sb[:], wv_d.rearrange("(cc p) c -> p cc c", p=128))
            wo_sb = persist.tile([128, NC_CHUNKS, C], bf16, tag="wo", name="wo")
            nc.sync.dma_start(wo_sb[:], wo_d.rearrange("(cc p) c -> p cc c", p=128))

            wq_sb = persist.tile([128, NC_CHUNKS, C], bf16, tag="wq", name="wq")
            nc.scalar.dma_start(
                wq_sb[:, :, 0:128], wqa_d.rearrange("(cc p) c -> p cc c", p=128)
            )
            nc.scalar.dma_start(
                wq_sb[:, :, 128:C], wqb_d.rearrange("(cc p) c -> p cc c", p=128)
            )

            cst = persist.tile([128, 146], f32, tag="cst", name="cst")
            nc.gpsimd.dma_start(cst[:], cst_d[:, :])
            wk_sb = persist.tile([128, NC_CHUNKS, C], bf16, tag="wk", name="wk")
            nc.gpsimd.dma_start(wk_sb[:], wk_d.rearrange("(cc p) c -> p cc c", p=128))

            tri = cst[:, 18:146].bitcast(bf16)  # [128, 256]; [:,0:128] = mask

            # ---- on-chip constants -----------------------------------
            # vt[sq][k_local, h, 0:64] = V'[c', s]^T ; col 64 = 1.0 (fused
            # softmax-denominator column). sel2 = 0/1 selector for the K=2
            # normalize broadcast (row p lights up partition half p).
            vt = []
            for sq in range(3):
                t = persist.tile([128, H, D + 1], bf16, tag=f"vt{sq}", name=f"vt{sq}")
                nc.gpsimd.memset(t[:, :, D : D + 1], 1.0)
                vt.append(t)
            sel2 = persist.tile([2, 128], f32r, tag="sel2", name="sel2")
            nc.gpsimd.dma_start(sel2[:], sel_d[:, :].bitcast(f32r))

            # ---- persistent working tiles ----------------------------
            q_sb = [
                persist.tile([128, S], bf16, tag=f"q{oc}", name=f"q{oc}")
                for oc in range(NC_CHUNKS)
            ]
            k_sb = [
                persist.tile([128, S], bf16, tag=f"k{oc}", name=f"k{oc}")
                for oc in range(NC_CHUNKS)
            ]
            attn_sb = [
                persist.tile([128, S], bf16, tag=f"at{oc}", name=f"at{oc}")
                for oc in range(NC_CHUNKS)
            ]
            # unnormalized attn, grouped by chunk: g0 = oc0-3, g1 = oc4-5
            AU = [
                persist.tile([128, 4 * S], f32, tag="au0", name="au0"),
                persist.tile([128, 2 * S], f32, tag="au1", name="au1"),
            ]
            # denominator staging (one partition) + reciprocal workspaces
            SU = persist.tile([1, H, S], f32, tag="su", name="su")
            sums_g = [
                persist.tile([24, 128], f32, tag="sm0", name="sm0"),
                persist.tile([12, 128], f32, tag="sm1", name="sm1"),
            ]
            rinv_g = [
                persist.tile([24, 128], f32, tag="ri0", name="ri0"),
                persist.tile([12, 128], f32, tag="ri1", name="ri1"),
            ]
            rr_g = [
                persist.tile([2, 4, S], f32r, tag="rr0", name="rr0"),
                persist.tile([2, 2, S], f32r, tag="rr1", name="rr1"),
            ]
            ot = persist.tile([128, NC_CHUNKS, S], bf16, tag="ot", name="ot")

            # ---- stage helpers ---------------------------------------
            def qkproj(oc, w_sb, bcol, out_sb):
                ps = psA.tile([128, S], f32, tag="proj", name="proj")
                for cc in range(NC_CHUNKS):
                    nc.tensor.matmul(
                        ps[:],
                        w_sb[:, cc, oc * 128 : (oc + 1) * 128],
                        xt[:, cc, :],
                        start=(cc == 0),
                        stop=(cc == NC_CHUNKS - 1),
                    )
                nc.vector.tensor_scalar_add(out_sb[:], ps[:], cst[:, bcol : bcol + 1])

            def vproj(sq, half):
                ps = psA.tile([128, S], f32, tag="proj", name="proj")
                for cc in range(NC_CHUNKS):
                    nc.tensor.matmul(
                        ps[:],
                        xt[:, cc, sq * 128 : (sq + 1) * 128],
                        wv_sb[:, cc, half * 384 : (half + 1) * 384],
                        start=(cc == 0),
                        stop=(cc == NC_CHUNKS - 1),
                    )
                nc.vector.tensor_copy(
                    vt[sq][:, half * 6 : (half + 1) * 6, 0:D],
                    ps[:].rearrange("p (h d) -> p h d", d=D),
                )

            def scores(h):
                # psum tile A: [kc0 | kc1] (cols 0:128 = q0:128 over keys
                # 0:128; cols 128:384 = q0:256 over keys 128:256), tile B:
                # kc2 q0:384. exp straight from PSUM; 0/1 triangle applied
                # after on the three diagonal sub-blocks (gpsimd, SBUF).
                oc, prow = h // 2, (h % 2) * D
                Qh = q_sb[oc][prow : prow + D, :]
                Kh = k_sb[oc][prow : prow + D, :]
                psa = psS.tile([128, S], f32, tag="sc", name="sc")
                nc.tensor.matmul(
                    psa[:, 0:128], Kh[:, 0:128], Qh[:, 0:128],
                    start=True, stop=True, skip_group_check=True,
                )
                nc.tensor.matmul(
                    psa[:, 128:384], Kh[:, 128:256], Qh[:, 0:256],
                    start=True, stop=True, skip_group_check=True,
                )
                psb = psS.tile([128, S], f32, tag="sc", name="sc")
                nc.tensor.matmul(
                    psb[:], Kh[:, 256:384], Qh[:, 0:384], start=True, stop=True,
                )
                eA = epool.tile([128, S], bf16, tag="eA", name="eA")
                nc.scalar.activation(eA[:], psa[:], Exp)
                eB = epool.tile([128, S], bf16, tag="eB", name="eB")
                nc.scalar.activation(eB[:], psb[:], Exp)
                nc.gpsimd.tensor_mul(eA[:, 0:128], eA[:, 0:128], tri[:, 0:128])
                nc.gpsimd.tensor_mul(eA[:, 256:384], eA[:, 256:384], tri[:, 0:128])
                nc.gpsimd.tensor_mul(eB[:, 256:384], eB[:, 256:384], tri[:, 0:128])
                return eA, eB

            def av(h, eA, eB):
                # accumulate widest first so every element's first write
                # carries the start flag
                oc, prow = h // 2, (h % 2) * D
                g, blk = (0, oc) if h < 8 else (1, oc - 4)
                ps_av = psV.tile([D + 1, S], f32, tag="av", name="av")
                nc.tensor.matmul(
                    ps_av[:, 0:384], vt[2][:, h, :], eB[:, 0:384],
                    start=True, stop=False, skip_group_check=True,
                )
                nc.tensor.matmul(
                    ps_av[:, 0:256], vt[1][:, h, :], eA[:, 128:384],
                    start=False, stop=False, skip_group_check=True,
                )
                nc.tensor.matmul(
                    ps_av[:, 0:128], vt[0][:, h, :], eA[:, 0:128],
                    start=False, stop=True, skip_group_check=True,
                )
                nc.vector.tensor_copy(
                    AU[g][prow : prow + D, blk * S : (blk + 1) * S], ps_av[0:D, :]
                )
                # SU column order is parity-major within each group so the
                # sums-gather DMA is one contiguous <=3-dim pattern
                su_idx = (
                    (h % 2) * 4 + h // 2 if h < 8 else 8 + (h % 2) * 2 + (h - 8) // 2
                )
                nc.scalar.activation(SU[0:1, su_idx, :], ps_av[D : D + 1, :], Copy)

            def normalize_group(g, oc0, n_oc):
                # sums -> [3n,128] parity-major rows (reciprocal is
                # free-dim bound, so spread over partitions) -> repack each
                # parity's rows to one partition -> per chunk one K=2
                # broadcast matmul + one full-chunk DVE multiply.
                h0, n_h = 2 * oc0, 2 * n_oc
                nc.sync.dma_start(sums_g[g][:], SU[0:1, h0 : h0 + n_h, :])
                nc.vector.reciprocal(rinv_g[g][:], sums_g[g][:])
                half = 3 * n_oc
                for p in range(2):
                    nc.sync.dma_start(
                        rr_g[g][p : p + 1, :, :],
                        rinv_g[g][p * half : (p + 1) * half, :].bitcast(f32r),
                    )
                for j in range(n_oc):
                    oc = oc0 + j
                    ps_r = psS.tile([128, S], f32, tag="sc", name="sc")
                    nc.tensor.matmul(
                        ps_r[:], sel2[:], rr_g[g][:, j, :], start=True, stop=True,
                    )
                    nc.vector.tensor_mul(
                        attn_sb[oc][:],
                        AU[g][:, j * S : (j + 1) * S],
                        ps_r[:],
                    )

            def oproj(oc):
                ps = psA.tile([128, S], f32, tag="proj", name="proj")
                for cc in range(NC_CHUNKS):
                    nc.tensor.matmul(
                        ps[:],
                        wo_sb[:, cc, oc * 128 : (oc + 1) * 128],
                        attn_sb[cc],
                        start=(cc == 0),
                        stop=(cc == NC_CHUNKS - 1),
                    )
                nc.scalar.activation(
                    ot[:, oc, :], ps[:], Ident, bias=cst[:, 12 + oc : 13 + oc]
                )
                if oc == 2:
                    nc.sync.dma_start(
                        y_d[0:384, :].rearrange("(cc p) s -> p cc s", p=128),
                        ot[:, 0:3, :],
                    )
                if oc == 5:
                    nc.sync.dma_start(
                        y_d[384:768, :].rearrange("(cc p) s -> p cc s", p=128),
                        ot[:, 3:6, :],
                    )

            # ---- schedule --------------------------------------------
            # Projections/scores for oc0-1 run while wv loads; V-proj
            # next; thereafter attn@V of chunk oc-2 leads each chunk's
            # projections so the Scalar exp latency is hidden behind
            # PE work that doesn't depend on it.
            e_tiles = {}
            for oc in (0, 1):
                qkproj(oc, wq_sb, oc, q_sb[oc])
                qkproj(oc, wk_sb, 6 + oc, k_sb[oc])
                for h in (2 * oc, 2 * oc + 1):
                    e_tiles[h] = scores(h)
            for sq in range(3):
                for half in range(2):
                    vproj(sq, half)
            for oc in (2, 3, 4, 5):
                for h in (2 * oc - 4, 2 * oc - 3):
                    av(h, *e_tiles.pop(h))
                qkproj(oc, wq_sb, oc, q_sb[oc])
                qkproj(oc, wk_sb, 6 + oc, k_sb[oc])
                for h in (2 * oc, 2 * oc + 1):
                    e_tiles[h] = scores(h)
            for h in (8, 9, 10, 11):
                av(h, *e_tiles.pop(h))
            normalize_group(0, 0, 4)
            normalize_group(1, 4, 2)
            for oc in range(NC_CHUNKS):
                oproj(oc)

    return nc


def _get_nc():
    if "nc" not in _STATE:
        _STATE["nc"] = _build_nc()
    return _STATE["nc"]


# --------------------------------------------------------------------------
def _prep_maps(inputs):
    import ml_dtypes

    bf16 = ml_dtypes.bfloat16

    hs = np.asarray(inputs["hidden_states"], dtype=np.float32)
    Wq = np.asarray(inputs["Wq"], dtype=np.float32)
    bq = np.asarray(inputs["bq"], dtype=np.float32)
    Wk = np.asarray(inputs["Wk"], dtype=np.float32)
    bk = np.asarray(inputs["bk"], dtype=np.float32)
    Wv = np.asarray(inputs["Wv"], dtype=np.float32)
    bv = np.asarray(inputs["bv"], dtype=np.float32)
    Wo = np.asarray(inputs["Wo"], dtype=np.float32)

    # head-major channel permutation: c' = h*64 + d  <-  c = d*12 + h
    idx = (np.arange(H)[:, None] + np.arange(D)[None, :] * H).reshape(C)
    scale = float(D) ** -0.5

    wqt = np.ascontiguousarray((scale * Wq[idx, :]).T).astype(bf16)
    wkt = np.ascontiguousarray(Wk[idx, :].T).astype(bf16)
    wvt = np.ascontiguousarray(Wv[idx, :].T).astype(bf16)
    wot = np.ascontiguousarray(Wo.T).astype(bf16)

    # packed constants [128, 146] f32
    cstf = np.zeros((128, 146), dtype=np.float32)
    cstf[:, 0:6] = (scale * bq[idx]).reshape(NC_CHUNKS, 128).T
    cstf[:, 6:12] = bk[idx].reshape(NC_CHUNKS, 128).T
    # V-bias folded through attention (softmax rows sum to 1):
    # out += Wo @ bv'
    cstf[:, 12:18] = (Wo @ bv[idx]).reshape(NC_CHUNKS, 128).T
    # 0/1 triangle: allowed keys are k >= q -> tri[k, q] = 1 iff k >= q
    tri = (
        np.tril(np.ones((128, 128), dtype=np.float32))
        .astype(bf16)
    )
    cstf[:, 18:146] = np.tile(tri, (1, 2)).view(np.float32)

    sel = np.zeros((2, 128), dtype=np.float32)
    sel[0, 0:64] = 1.0
    sel[1, 64:128] = 1.0

    shared = {
        "wqa": np.ascontiguousarray(wqt[:, 0:128]),
        "wqb": np.ascontiguousarray(wqt[:, 128:C]),
        "wkt": wkt, "wvt": wvt, "wot": wot,
        "cst": cstf, "sel": sel,
    }
    maps = []
    for b in range(B):
        xb = np.ascontiguousarray(hs[b, :, 0, :]).astype(bf16)
        maps.append({"x": xb, **shared})
    return maps


def _run(inputs, trace=False, **kwargs):
    from concourse.bass_utils import run_bass_kernel_spmd

    nc = _get_nc()
    in_maps = _prep_maps(inputs)
    res = run_bass_kernel_spmd(
        nc, in_maps, core_ids=list(range(B)), trace=trace, **kwargs
    )
    out = np.stack(
        [np.asarray(res.results[b]["y"]).astype(np.float32) for b in range(B)],
        axis=0,
    )
    return out.reshape(B, C, 1, S), res


def kernel(**inputs):
    out, _ = _run(inputs, trace=False)
    return out
